# revision 48
# baseline (speedup 1.0000x reference)
"""Trainium2 Bass kernel for nn_MultiHeadDiffAttention (B=2,T=2048,C=1024,H=16).

Sharding: 8 cores = data-parallel over B(2) x tensor-parallel over 4 head-groups
(4 heads each). Each core computes q/k/v projections for its heads, causal
differential attention, per-head GroupNorm, and a partial output projection
(its 512 columns of y2 against Wc). Host sums the 4 partials per batch.

v3 schedule (over the v2 layout):
  - x DMA'd first across all 4 issue queues; q1/k1 then q2/k2 weights next,
    so attention S-matmuls start ~30us in instead of ~60us.
  - only the oc=0 (heads 0-1) q/k projections run up front; oc=1 and the v
    projection drain as background PE work inside the head-0/1 loops.
  - head pair (2,3) has no projection work left to fill PE gaps (its phase
    is ACT-exp-bound), so the j=0/1 half of the output projection runs
    there instead: partial (y0*Wc0 + y1*Wc1) per ocb into bf16 SBUF tiles
    (space freed by closing the loads pool at the pair boundary), added
    back in the final phase via an identity-stationary matmul.
  - combine att' = e1 + s*e2 split into tensor_scalar (4x DVE mode) +
    tensor_tensor (2x) instead of one 1x scalar_tensor_tensor pass.
  - psum->sbuf evictions rotate across DVE/ACT/GPSIMD so neither ACT (exp)
    nor DVE (combine/casts) eats them all.
  - outT is DMA'd in bf16 (halves the output traffic); host upcasts.
"""

import sys

for _p in ("/opt/trn_rl_repo", "/root/.axon_site/_ro/trn_rl_repo"):
    if _p not in sys.path:
        sys.path.insert(0, _p)

import math
import numpy as np
import ml_dtypes

import concourse.bass as bass
import concourse.bacc as bacc
import concourse.tile as tile
import concourse.mybir as mybir
from concourse import bass_utils

F32 = mybir.dt.float32
BF16 = mybir.dt.bfloat16
AF = mybir.ActivationFunctionType
ALU = mybir.AluOpType

B, T, C = 2, 2048, 1024
H = 16
HS = C // H           # 64
D = 2 * HS            # 128 v-channels per head
NH = 4                # heads per core
N_CORES = 8
NT = T // 128         # 16 q-tiles
LAMBDA_INIT = 0.8 - 0.6 * math.exp(-0.3 * (12 - 1))
EPS = 1e-5
SCALE = 1.0 / math.sqrt(HS)
NEG = -30000.0

_cache = {}


def _build(T=T, trace_sim=False, nh=NH):
    NT = T // 128
    nc = bacc.Bacc("TRN2", target_bir_lowering=False, debug=False,
                   num_devices=N_CORES)

    def din(name, shape, dt=BF16):
        return nc.dram_tensor(name, shape, dt, kind="ExternalInput").ap()

    xT_d = din("xT", [C, T])
    # q/k/v weights arrive in host-merged tile layout: [2 tiles x 128
    # partitions, 4 chunks x cols] so each needs only a few DMAs
    wq1_d = din("wq1T", [256, 4 * NH * HS])
    wq2_d = din("wq2T", [256, 4 * NH * HS])
    wk1_d = din("wk1T", [256, 4 * NH * HS])
    wk2_d = din("wk2T", [256, 4 * NH * HS])
    wv_d = din("wvT", [256, 4 * NH * D])
    wc_d = din("wcT", [NH * D, C])
    # merged constants: one f32 blob (gg | gw2 | gb2 | icon-bits | lamn) and
    # one bf16 blob (mask | ident) -> 2 DMA issues instead of 7
    cstf_d = din("cstf", [128, 136], F32)
    cstb_d = din("cstb", [128, 256])
    outT_d = nc.dram_tensor("outT", [C, T], BF16, kind="ExternalOutput").ap()
    outT2_d = nc.dram_tensor("outT2", [C, T], BF16, kind="ExternalOutput").ap()

    with tile.TileContext(nc, trace_sim=trace_sim) as tc:
        with tc.tile_pool(name="persist", bufs=1) as pp, \
             tc.tile_pool(name="ps_s", bufs=3, space="PSUM") as ps_s, \
             tc.tile_pool(name="ps_t", bufs=2, space="PSUM") as ps_t:

            # ---- PE warmup: ramp the tensor-engine P-state during the
            # input DMA window with dummy back-to-back matmuls ----
            wa_t = pp.tile([128, 128], BF16, tag="wa")
            nc.vector.memset(wa_t[:], 0.0)
            for _ in range(16):
                psw = ps_s.tile([128, 1024], F32, tag="s")
                nc.tensor.matmul(psw[:, 0:128], wa_t[:], wa_t[:],
                                 start=True, stop=True)

            # ---- persistent small tiles (merged blobs) ----
            cstf = pp.tile([128, 136], F32, tag="cstf")
            cstb = pp.tile([128, 256], BF16, tag="cstb")
            gg_t = cstf[:, 0:128]
            gw2_t = cstf[:, 128:129]
            gb2_t = cstf[:, 129:130]
            icon0 = cstf.bitcast(mybir.dt.uint32)[:, 130:131]
            icon1 = cstf.bitcast(mybir.dt.uint32)[:, 131:132]
            mask_t = cstb[:, 0:128]
            ident_t = cstb[:, 128:256]

            def lamn_sl(j):
                return cstf[:, 132 + j:133 + j]
            c15_t = pp.tile([128, 1], F32, tag="c15")
            nc.vector.memset(c15_t[:], 1.5)

            # ---- persistent activation tensors ----
            q1t = [pp.tile([128, T], BF16, tag=f"q1t{i}", name=f"q1t{i}") for i in range(2)]
            q2t = [pp.tile([128, T], BF16, tag=f"q2t{i}", name=f"q2t{i}") for i in range(2)]
            k1t = [pp.tile([128, T], BF16, tag=f"k1t{i}", name=f"k1t{i}") for i in range(2)]
            k2t = [pp.tile([128, T], BF16, tag=f"k2t{i}", name=f"k2t{i}") for i in range(2)]
            vt = [pp.tile([128, NH * D], BF16, tag=f"vt{i}", name=f"vt{i}") for i in range(NT)]
            # yT per head [D=128, T] bf16 (post-groupnorm)
            yt = [pp.tile([128, T], BF16, tag=f"yt{j}", name=f"yt{j}") for j in range(NH)]
            # wcT: [512, C] as 4 f-chunk tiles (one per head)
            wct = [pp.tile([128, C], BF16, tag=f"wct{j}", name=f"wct{j}") for j in range(NH)]

            # attention working pool opened BEFORE the loads pool so the
            # loads pool can be released first (pools close LIFO)
            wp_cm = tc.tile_pool(name="aw", bufs=2)
            wp = wp_cm.__enter__()

            # ================= q/k projection loads =================
            lp_cm = tc.tile_pool(name="loads", bufs=1)
            lp = lp_cm.__enter__()
            xt = [lp.tile([128, T], BF16, tag=f"xt{i}", name=f"xt{i}") for i in range(8)]
            wqm = {}
            for nm in ("q1", "q2", "k1", "k2"):
                wqm[nm] = [lp.tile([128, 4 * NH * HS], BF16,
                                   tag=f"w{nm}{h}", name=f"w{nm}{h}")
                           for h in range(2)]
            wvm = [lp.tile([128, 4 * NH * D], BF16, tag=f"wvm{h}",
                           name=f"wvm{h}") for h in range(2)]

            def wq_sl(nm, cc, oc):
                return wqm[nm][cc // 4][:, (cc % 4) * 256 + oc * 128:
                                        (cc % 4) * 256 + oc * 128 + 128]

            def wv_sl(cc):
                return wvm[cc // 4][:, (cc % 4) * 512:(cc % 4) * 512 + 512]

            # ---- DMA schedule (3 issue queues: SP/ACT/Pool): x chunks 0-3
            # and the h=0 halves of q1/k1/q2/k2 first (the first 4 cc-chunks
            # of each projection can start on those), then x4-7 + h=1 ----
            wdsc = {"q1": wq1_d, "k1": wk1_d, "q2": wq2_d, "k2": wk2_d}

            def w_dma(eng, nm, h):
                eng.dma_start(wqm[nm][h][:],
                              wdsc[nm][h * 128:(h + 1) * 128, :])

            def x_dma(eng, i):
                eng.dma_start(xt[i][:], xT_d[i * 128:(i + 1) * 128, :])

            # weights are small (256KB per half) and are needed by every
            # x-chunk matmul: land them first, then stream x in cc order
            w_dma(nc.sync, "q1", 0)
            w_dma(nc.scalar, "k1", 0)
            w_dma(nc.gpsimd, "q2", 0)
            w_dma(nc.sync, "q1", 1)
            w_dma(nc.scalar, "k1", 1)
            w_dma(nc.gpsimd, "k2", 0)
            x_dma(nc.sync, 0)
            x_dma(nc.scalar, 1)
            x_dma(nc.gpsimd, 2)
            x_dma(nc.sync, 3)
            x_dma(nc.scalar, 4)
            x_dma(nc.gpsimd, 5)
            x_dma(nc.sync, 6)
            x_dma(nc.scalar, 7)
            w_dma(nc.gpsimd, "q2", 1)
            w_dma(nc.gpsimd, "k2", 1)
            nc.gpsimd.dma_start(cstb[:], cstb_d)
            # wv needed by the first background v chunks (~35us in)
            nc.sync.dma_start(wvm[0][:], wv_d[0:128, :])
            nc.scalar.dma_start(wvm[1][:], wv_d[128:256, :])
            # wc + groupnorm consts needed late
            for j in range(NH):
                eng = (nc.sync, nc.scalar)[j % 2]
                eng.dma_start(wct[j][:], wc_d[j * 128:(j + 1) * 128, :])
            nc.gpsimd.dma_start(cstf[:], cstf_d)

            # qT/kT projections: out [o=128, t=512] = W^T_chunk.T @ xT
            _qk_ps = {}

            def emit_qk(nm, dst, oc, half):
                if half == 0:
                    _qk_ps[(nm, oc)] = (ps_s.tile([128, 1024], F32, tag="s", name="qkA"),
                                        ps_s.tile([128, 1024], F32, tag="s", name="qkB"))
                psA, psB = _qk_ps[(nm, oc)]
                for cc in range(4 * half, 4 * half + 4):
                    for tb in range(T // 512):
                        ph = (psA, psB)[tb // 2]
                        mi = nc.tensor.matmul(
                            ph[:, (tb % 2) * 512:(tb % 2) * 512 + 512],
                            wq_sl(nm, cc, oc),
                            xt[cc][:, tb * 512:(tb + 1) * 512],
                            start=(cc == 0), stop=(cc == 7),
                            skip_group_check=True)
                        if tb > 0:
                            mi.ins.ldweights = False
                if half == 1:
                    for tb2 in range(2):
                        src = (psA, psB)[tb2][:]
                        if nm in ("q1", "k1"):
                            nc.scalar.copy(
                                dst[oc][:, tb2 * 1024:(tb2 + 1) * 1024], src)
                        else:
                            nc.vector.tensor_copy(
                                dst[oc][:, tb2 * 1024:(tb2 + 1) * 1024], src)

            def emit_qk_full(nm, dst, oc):
                emit_qk(nm, dst, oc, 0)
                emit_qk(nm, dst, oc, 1)

            # oc=0 projections up front (heads 0-1 attention needs them)
            for nm, dst in (("q1", q1t), ("k1", k1t), ("q2", q2t), ("k2", k2t)):
                emit_qk_full(nm, dst, 0)

            # v projection chunks are emitted lazily
            v_done = [False] * NT

            def emit_v(tch):
                if v_done[tch]:
                    return
                v_done[tch] = True
                ps = ps_t.tile([128, NH * D], F32, tag="tz")
                for cc in range(8):
                    nc.tensor.matmul(
                        ps[:],
                        xt[cc][:, tch * 128:(tch + 1) * 128],
                        wv_sl(cc),
                        start=(cc == 0), stop=(cc == 7))
                if tch % 2 == 0:
                    nc.vector.tensor_copy(vt[tch][:], ps[:])
                else:
                    nc.scalar.copy(vt[tch][:], ps[:])

            # background PE work drained between attention phases of
            # heads 0-1: v chunks (just-in-time via the phase2 safety),
            # then the oc=1 projections (needed only by pair (2,3))
            bg = [(lambda t=t: emit_v(t)) for t in range(NT)]
            for nm, dst in (("q1", q1t), ("k1", k1t),
                            ("q2", q2t), ("k2", k2t)):
                bg.append(lambda nm=nm, dst=dst: emit_qk_full(nm, dst, 1))

            def bg_drain(n):
                for _ in range(min(n, len(bg))):
                    bg.pop(0)()

            # ================= attention per head =================
            head_ctx = {}

            def phase1_qt(j, qb, qq, AB):
                oc, po = divmod(j * HS, 128)
                qt = qb * 4 + qq
                nk = qt + 1
                nkb2 = (nk + 7) // 8   # 1024-wide S psum tiles
                e1 = wp.tile([128, T], BF16, tag="e1", name="e1", bufs=4)
                e2 = wp.tile([128, T], BF16, tag="e2", name="e2", bufs=3)
                dd = wp.tile([128, 2], F32, tag="dd", name="dd", bufs=4)
                d1c = wp.tile([128, 2], F32, tag="d1c", name="d1c", bufs=4)
                d2c = wp.tile([128, 2], F32, tag="d2c", name="d2c", bufs=4)
                for mi, (qsrc, ksrc, erow, dcol) in enumerate(
                        ((q1t, k1t, e1, d1c), (q2t, k2t, e2, d2c))):
                    for kb in range(nkb2):
                        w = min(1024, nk * 128 - kb * 1024)
                        ps = ps_s.tile([128, 1024], F32, tag="s",
                                       name="ps")
                        off = qt * 128 - kb * 1024  # diag block col
                        for hf in range(2):
                            wh = min(512, w - hf * 512)
                            if wh <= 0:
                                break
                            diag_here = (kb == nkb2 - 1 and
                                         hf * 512 <= off < hf * 512 + wh)
                            mm = nc.tensor.matmul(
                                ps[:, hf * 512:hf * 512 + wh],
                                qsrc[oc][po:po + HS,
                                         qt * 128:(qt + 1) * 128],
                                ksrc[oc][po:po + HS,
                                         kb * 1024 + hf * 512:
                                         kb * 1024 + hf * 512 + wh],
                                start=True, stop=not diag_here,
                                skip_group_check=diag_here)
                            if kb + hf > 0:
                                mm.ins.ldweights = False
                        if kb == nkb2 - 1:
                            # mask diagonal 128-block on PE
                            nc.tensor.matmul(
                                ps[:, off:off + 128],
                                ident_t, mask_t,
                                start=False, stop=True,
                                skip_group_check=True)
                        # accum straight into dd when a single psum covers
                        # the row (saves two DVE copies per qt)
                        acc = (dd[:, mi:mi + 1] if nkb2 == 1
                               else dcol[:, kb:kb + 1])
                        nc.scalar.activation(
                            erow[:, kb * 1024:kb * 1024 + w],
                            ps[:, :w], AF.Exp, scale=SCALE,
                            accum_out=acc)
                # denominators -> rr = [1/D1, 1/D2]
                rr = wp.tile([128, 2], F32, tag="rr", name="rr", bufs=4)
                if nkb2 > 1:
                    nc.vector.tensor_reduce(
                        dd[:, 0:1], d1c[:, 0:nkb2],
                        axis=mybir.AxisListType.X, op=ALU.add)
                    nc.vector.tensor_reduce(
                        dd[:, 1:2], d2c[:, 0:nkb2],
                        axis=mybir.AxisListType.X, op=ALU.add)
                nc.vector.reciprocal(rr[:], dd[:, 0:2])
                # sc2 = -lam / D2 (per-partition scalar)
                sc2 = wp.tile([128, 1], F32, tag="sc2", name="sc2", bufs=4)
                nc.vector.tensor_tensor(sc2[:], rr[:, 1:2],
                                        lamn_sl(j), ALU.mult)
                # att = e1*r1 + e2*sc2 (fully normalized diff-attention
                # row): tensor_scalar (4x) + scalar_tensor_tensor
                etmp = wp.tile([128, T], BF16, tag="etmp", name="etmp",
                               bufs=1)
                nc.vector.tensor_scalar(
                    etmp[:, :nk * 128], e2[:, :nk * 128], sc2[:, 0:1],
                    None, op0=ALU.mult)
                nc.vector.scalar_tensor_tensor(
                    e1[:, :nk * 128], e1[:, :nk * 128], rr[:, 0:1],
                    etmp[:, :nk * 128], op0=ALU.mult, op1=ALU.add)
                # transpose this attention row on the DMA xbar:
                # [q=128, nk*128] -> nk transposed blocks [k=128, 128]
                nc.sync.dma_start_transpose(
                    AB[:, 0:nk, qq, :], e1[:, :nk * 128])

            def phase1_pair(jA, jB, qb):
                # qt-interleaved emission across the head pair: the PE always
                # has the other head's independent S-chunk while ACT/DVE
                # drain this one's exp/combine chain
                ABs = {}
                for j in (jA, jB):
                    # AB layout [k=128, kc, qq, q-col]: z-matmul rhs slices
                    # contiguous (strided moving operands are silently wrong
                    # on HW; strided DMA-transpose OUT is fine)
                    ABs[j] = wp.tile([128, NT, 4, 128], BF16, tag="AB",
                                     name="AB", bufs=2)
                for qq in range(4):
                    for j in (jA, jB):
                        phase1_qt(j, qb, qq, ABs[j])
                return ABs

            def phase2(j, qb, AB):
                ytr, s1p, s2p = head_ctx[j]
                nkc = qb * 4 + 4
                # yT[d, qblk] = sum_kc v_kc.T @ attT_kc   (N=512)
                py = ps_t.tile([128, 512], F32, tag="tz", name="py")
                for kc in range(nkc):
                    emit_v(kc)
                    qq0 = max(0, kc - qb * 4)
                    zw = qq0 * 128
                    nc.tensor.matmul(
                        py[:, zw:],
                        vt[kc][:, j * 128:(j + 1) * 128],
                        AB[:, kc, qq0:4, :].rearrange("p q c -> p (q c)"),
                        start=(kc == 0), stop=(kc == nkc - 1),
                        skip_group_check=True)
                # copy to ytr with fused stats accumulation on DVE (no
                # 183ns ACT accumulator-read tax; ACT is the busier engine)
                nc.vector.tensor_scalar(
                    ytr[:, qb * 512:(qb + 1) * 512], py[:], 1.0, 0.0,
                    op0=ALU.mult, op1=ALU.add,
                    accum_out=s1p[:, qb:qb + 1])
                ysq = wp.tile([128, 512], BF16, tag="ysq", name="ysq",
                              bufs=1)
                ysrc = ytr[:, qb * 512:(qb + 1) * 512]
                nc.vector.scalar_tensor_tensor(
                    ysq[:], ysrc, 1.0, ysrc,
                    op0=ALU.mult, op1=ALU.mult,
                    accum_out=s2p[:, qb:qb + 1])

            def gn_final(j):
                ytr, s1p, s2p = head_ctx[j]
                s12 = wp.tile([128, 2], F32, tag="s12", name="s12")
                nc.vector.tensor_reduce(s12[:, 0:1], s1p[:, 0:NT // 4],
                                        axis=mybir.AxisListType.X, op=ALU.add)
                nc.vector.tensor_reduce(s12[:, 1:2], s2p[:, 0:NT // 4],
                                        axis=mybir.AxisListType.X, op=ALU.add)
                pg = ps_t.tile([128, 2], F32, tag="tz", name="pg")
                nc.tensor.matmul(pg[:], gg_t, s12[:], start=True, stop=True)
                # mneg = -mean; nvar = mean^2 - E[y^2] = -var
                mneg = wp.tile([128, 1], F32, tag="mneg", name="mneg")
                nc.scalar.mul(mneg[:], pg[:, 0:1], -1.0 / (T * 4))
                msq = wp.tile([128, 1], F32, tag="msq")
                nc.scalar.mul(msq[:], pg[:, 1:2], 1.0 / (T * 4))
                nvar = wp.tile([128, 1], F32, tag="nvar")
                nc.vector.scalar_tensor_tensor(
                    nvar[:], mneg[:], mneg[:, 0:1], msq[:],
                    op0=ALU.mult, op1=ALU.subtract)
                vpe = wp.tile([128, 1], F32, tag="vpe")
                nc.vector.tensor_scalar(vpe[:], nvar[:], -1.0, EPS,
                                        op0=ALU.mult, op1=ALU.add)  # var+eps
                # rsqrt(var+eps) on DVE only: quake seed + Newton iters
                rstd = wp.tile([128, 1], F32, tag="rstd")
                yi = wp.tile([128, 1], F32, tag="yi")
                nc.vector.tensor_tensor(yi.bitcast(mybir.dt.uint32)[:],
                                        vpe.bitcast(mybir.dt.uint32)[:],
                                        icon0,
                                        ALU.logical_shift_right)
                nc.vector.tensor_tensor(yi.bitcast(mybir.dt.uint32)[:],
                                        icon1,
                                        yi.bitcast(mybir.dt.uint32)[:],
                                        ALU.subtract)
                vneg = wp.tile([128, 1], F32, tag="vneg")
                nc.vector.tensor_scalar_mul(vneg[:], vpe[:], -0.5)
                ytmp = wp.tile([128, 1], F32, tag="ytmp")
                for _ in range(2):
                    nc.vector.tensor_tensor(ytmp[:], yi[:], yi[:], ALU.mult)
                    nc.vector.scalar_tensor_tensor(
                        ytmp[:], ytmp[:], vneg[:, 0:1], c15_t[:],
                        op0=ALU.mult, op1=ALU.add)  # 1.5 - 0.5 v y^2
                    nc.vector.tensor_tensor(yi[:], yi[:], ytmp[:], ALU.mult)
                nc.vector.tensor_copy(rstd[:], yi[:])
                aff_a = wp.tile([128, 1], F32, tag="aff_a")
                nc.vector.tensor_tensor(aff_a[:], rstd[:], gw2_t, ALU.mult)
                aff_b = wp.tile([128, 1], F32, tag="aff_b")
                nc.vector.scalar_tensor_tensor(
                    aff_b[:], mneg[:], aff_a[:, 0:1], gb2_t,
                    op0=ALU.mult, op1=ALU.add)  # gb2 - mean*aff_a
                # affine on DVE (4x): yt = ytr*aff_a + aff_b
                nc.vector.tensor_scalar(yt[j][:], ytr[:], aff_a[:, 0:1],
                                        aff_b[:, 0:1],
                                        op0=ALU.mult, op1=ALU.add)

            def new_head(j):
                head_ctx[j] = (
                    wp.tile([128, T], BF16, tag="ytr", name="ytr", bufs=3),
                    wp.tile([128, 4], F32, tag="s1p", name="s1p"),
                    wp.tile([128, 4], F32, tag="s2p", name="s2p"))

            # qb order (1,2,3,0): the serial end-of-pair chain (last exp ->
            # combine -> transpose -> z -> stats -> gn) runs on the smallest
            # q-block, shrinking the pair-boundary latency
            QBS = (1, 2, 3, 0)

            # ---- pair (0,1): bg (v + oc1 projections) fills the PE ----
            for j in (0, 1):
                new_head(j)
            for qb in QBS:
                ABs = phase1_pair(0, 1, qb)
                bg_drain(5)
                phase2(0, qb, ABs[0])
                if qb == 0:
                    bg_drain(len(bg))
                    gn_final(0)
                phase2(1, qb, ABs[1])
                if qb == 0:
                    gn_final(1)

            # pair boundary: release x/weight tiles, open the partial pool
            lp_cm.__exit__(None, None, None)
            dp_cm = tc.tile_pool(name="drain", bufs=1)
            dp = dp_cm.__enter__()
            p01 = [dp.tile([128, T], BF16, tag=f"p01_{ocb}",
                           name=f"p01_{ocb}") for ocb in range(8)]

            # out-proj j=0,1 partials: fill PE during the ACT-bound
            # (2,3) pair. Two ocbs per qb slot.
            fill_q = list(range(8))

            def emit_fill(ocb):
                for tb in range(T // 512):
                    pt = ps_t.tile([128, 512], F32, tag="tz", name="fl")
                    for j in (0, 1):
                        nc.tensor.matmul(
                            pt[:],
                            wct[j][:, ocb * 128:(ocb + 1) * 128],
                            yt[j][:, tb * 512:(tb + 1) * 512],
                            start=(j == 0), stop=(j == 1),
                            skip_group_check=True)
                    nc.vector.tensor_copy(
                        p01[ocb][:, tb * 512:(tb + 1) * 512], pt[:])
                # ship the j01 partial to the host during the attention
                # phase (DMA engines are idle here); host adds the partials
                nc.gpsimd.dma_start(
                    outT2_d[ocb * 128:(ocb + 1) * 128, :], p01[ocb][:])

            # ---- pair (2,3): fills + attention ----
            for j in (2, 3):
                new_head(j)
            for qb in QBS:
                ABs = phase1_pair(2, 3, qb)
                for _ in range(2):
                    if fill_q:
                        emit_fill(fill_q.pop(0))
                phase2(2, qb, ABs[2])
                if qb == 0:
                    while fill_q:
                        emit_fill(fill_q.pop(0))
                    gn_final(2)
                phase2(3, qb, ABs[3])
                if qb == 0:
                    gn_final(3)

            # ================= output projection =================
            # per ocb: ident-add of the j01 partial, then j=2, then j=3.
            # gn(3) is emitted just before ocb0 so its ACT->DVE chain hides
            # behind the gn-independent ident-adds + j=2 matmuls.
            def s_halves():
                psA = ps_s.tile([128, 1024], F32, tag="s", name="psA")
                psB = ps_s.tile([128, 1024], F32, tag="s", name="psB")
                return (psA, psB)

            def op_mms23(ocb, halves):
                for j in (2, 3):
                    for tb in range(T // 512):
                        ph = halves[tb // 2]
                        mi = nc.tensor.matmul(
                            ph[:, (tb % 2) * 512:(tb % 2) * 512 + 512],
                            wct[j][:, ocb * 128:(ocb + 1) * 128],
                            yt[j][:, tb * 512:(tb + 1) * 512],
                            start=(j == 2), stop=(j == 3),
                            skip_group_check=True)
                        if tb > 0:
                            mi.ins.ldweights = False

            _fin_rr = [0]

            def op_fin(ocb, halves):
                for tb2 in range(2):
                    ob = dp.tile([128, 1024], BF16, tag="ob", bufs=4,
                                 name="ob")
                    r = _fin_rr[0] = (_fin_rr[0] + 1) % 2
                    if r == 0:
                        nc.vector.tensor_copy(ob[:], halves[tb2][:])
                    else:
                        nc.scalar.copy(ob[:], halves[tb2][:])
                    eng = (nc.sync, nc.gpsimd, nc.scalar)[(2 * ocb + tb2) % 3]
                    eng.dma_start(
                        outT_d[ocb * 128:(ocb + 1) * 128,
                               tb2 * 1024:(tb2 + 1) * 1024], ob[:])

            for ocb in range(8):
                halves = s_halves()
                op_mms23(ocb, halves)
                op_fin(ocb, halves)
            dp_cm.__exit__(None, None, None)
            wp_cm.__exit__(None, None, None)

    nc.compile()
    return nc


def _prep_inputs(inputs):
    bf = ml_dtypes.bfloat16
    x = np.asarray(inputs["x"], np.float32)
    Wq1 = np.asarray(inputs["Wq1"], np.float32)
    Wq2 = np.asarray(inputs["Wq2"], np.float32)
    Wk1 = np.asarray(inputs["Wk1"], np.float32)
    Wk2 = np.asarray(inputs["Wk2"], np.float32)
    Wv = np.asarray(inputs["Wv"], np.float32)
    Wc = np.asarray(inputs["Wc"], np.float32)
    gn_w = np.asarray(inputs["gn_w"], np.float32)
    gn_b = np.asarray(inputs["gn_b"], np.float32)
    gamma = np.asarray(inputs["gamma"], np.float32)

    def sig(v):
        return 1.0 / (1.0 + np.exp(-v))

    lam = (sig(np.asarray(inputs["lq1"], np.float32).reshape(H)
               * np.asarray(inputs["lk1"], np.float32).reshape(H))
           - sig(np.asarray(inputs["lq2"], np.float32).reshape(H)
                 * np.asarray(inputs["lk2"], np.float32).reshape(H))
           + LAMBDA_INIT)

    mask = np.where(np.arange(128)[None, :] <= np.arange(128)[:, None],
                    0.0, NEG).astype(bf)
    ident = np.eye(128, dtype=np.float32).astype(bf)
    gg = (np.arange(128)[:, None] // 4 == np.arange(128)[None, :] // 4
          ).astype(np.float32)
    c1 = 1.0 - LAMBDA_INIT
    gw2 = (gn_w * gamma * c1).astype(np.float32).reshape(128, 1)
    gb2 = (gn_b * gamma * c1).astype(np.float32).reshape(128, 1)

    icon = np.zeros((128, 2), np.uint32)
    icon[:, 0] = 1
    icon[:, 1] = 0x5f375a00
    cstb = np.concatenate([mask, ident], axis=1)
    xTb = [np.ascontiguousarray(x[b].T).astype(bf) for b in range(B)]
    in_maps = []
    for core in range(N_CORES):
        b, hg = divmod(core, N_CORES // B)
        qs = hg * NH * HS          # 256-wide q/k slice
        vs = hg * NH * D           # 512-wide v / y2 slice
        lamn = np.repeat(-lam[hg * NH:(hg + 1) * NH].reshape(1, NH),
                         128, axis=0).astype(np.float32)
        cstf = np.concatenate(
            [gg, gw2, gb2, icon.view(np.float32), lamn],
            axis=1).astype(np.float32)
        def mtiles(wt, cols):
            # [1024, cols] -> merged 2-tile layout [256, 4*cols]
            return np.ascontiguousarray(
                wt.reshape(2, 4, 128, cols).transpose(0, 2, 1, 3)
                .reshape(256, 4 * cols)).astype(bf)

        in_maps.append({
            "xT": xTb[b],
            "wq1T": mtiles(Wq1[qs:qs + NH * HS, :].T, NH * HS),
            "wq2T": mtiles(Wq2[qs:qs + NH * HS, :].T, NH * HS),
            "wk1T": mtiles(Wk1[qs:qs + NH * HS, :].T, NH * HS),
            "wk2T": mtiles(Wk2[qs:qs + NH * HS, :].T, NH * HS),
            "wvT": mtiles(Wv[vs:vs + NH * D, :].T, NH * D),
            "wcT": np.ascontiguousarray(Wc[:, vs:vs + NH * D].T).astype(bf),
            "cstf": cstf,
            "cstb": cstb,
        })
    return in_maps


def kernel(**inputs):
    if "nc" not in _cache:
        _cache["nc"] = _build()
    nc = _cache["nc"]
    in_maps = _prep_inputs(inputs)
    res = bass_utils.run_bass_kernel_spmd(
        nc, in_maps, core_ids=list(range(N_CORES)),
        **_cache.get("run_kwargs", {}))
    _cache["last_result"] = res
    out = np.zeros((B, T, C), np.float32)
    for core in range(N_CORES):
        b = core // (N_CORES // B)
        out[b] += res.results[core]["outT"].T.astype(np.float32)
        out[b] += res.results[core]["outT2"].T.astype(np.float32)
    return out


# revision 49
# speedup vs baseline: 1.0451x; 1.0451x over previous
"""Trainium2 Bass kernel for nn_MultiHeadDiffAttention (B=2,T=2048,C=1024,H=16).

Sharding: 8 cores = data-parallel over B(2) x tensor-parallel over 4 head-groups
(4 heads each). Each core computes q/k/v projections for its heads, causal
differential attention, per-head GroupNorm, and a partial output projection
(its 512 columns of y2 against Wc). Host sums the 4 partials per batch.

v3 schedule (over the v2 layout):
  - x DMA'd first across all 4 issue queues; q1/k1 then q2/k2 weights next,
    so attention S-matmuls start ~30us in instead of ~60us.
  - only the oc=0 (heads 0-1) q/k projections run up front; oc=1 and the v
    projection drain as background PE work inside the head-0/1 loops.
  - head pair (2,3) has no projection work left to fill PE gaps (its phase
    is ACT-exp-bound), so the j=0/1 half of the output projection runs
    there instead: partial (y0*Wc0 + y1*Wc1) per ocb into bf16 SBUF tiles
    (space freed by closing the loads pool at the pair boundary), added
    back in the final phase via an identity-stationary matmul.
  - combine att' = e1 + s*e2 split into tensor_scalar (4x DVE mode) +
    tensor_tensor (2x) instead of one 1x scalar_tensor_tensor pass.
  - psum->sbuf evictions rotate across DVE/ACT/GPSIMD so neither ACT (exp)
    nor DVE (combine/casts) eats them all.
  - outT is DMA'd in bf16 (halves the output traffic); host upcasts.
"""

import sys

for _p in ("/opt/trn_rl_repo", "/root/.axon_site/_ro/trn_rl_repo"):
    if _p not in sys.path:
        sys.path.insert(0, _p)

import math
import numpy as np
import ml_dtypes

import concourse.bass as bass
import concourse.bacc as bacc
import concourse.tile as tile
import concourse.mybir as mybir
from concourse import bass_utils

F32 = mybir.dt.float32
BF16 = mybir.dt.bfloat16
AF = mybir.ActivationFunctionType
ALU = mybir.AluOpType

B, T, C = 2, 2048, 1024
H = 16
HS = C // H           # 64
D = 2 * HS            # 128 v-channels per head
NH = 4                # heads per core
N_CORES = 8
NT = T // 128         # 16 q-tiles
LAMBDA_INIT = 0.8 - 0.6 * math.exp(-0.3 * (12 - 1))
EPS = 1e-5
SCALE = 1.0 / math.sqrt(HS)
NEG = -30000.0

_cache = {}


def _build(T=T, trace_sim=False, nh=NH):
    NT = T // 128
    nc = bacc.Bacc("TRN2", target_bir_lowering=False, debug=False,
                   num_devices=N_CORES)

    def din(name, shape, dt=BF16):
        return nc.dram_tensor(name, shape, dt, kind="ExternalInput").ap()

    xT_d = din("xT", [C, T])
    # q/k/v weights arrive in host-merged tile layout: [2 tiles x 128
    # partitions, 4 chunks x cols] so each needs only a few DMAs
    wq1_d = din("wq1T", [256, 4 * NH * HS])
    wq2_d = din("wq2T", [256, 4 * NH * HS])
    wk1_d = din("wk1T", [256, 4 * NH * HS])
    wk2_d = din("wk2T", [256, 4 * NH * HS])
    wv_d = din("wvT", [256, 4 * NH * D])
    wc_d = din("wcT", [NH * D, C])
    # merged constants: one f32 blob (gg | gw2 | gb2 | icon-bits | lamn) and
    # one bf16 blob (mask | ident) -> 2 DMA issues instead of 7
    cstf_d = din("cstf", [128, 136], F32)
    cstb_d = din("cstb", [128, 256])
    outT_d = nc.dram_tensor("outT", [C, T], BF16, kind="ExternalOutput").ap()
    outT2_d = nc.dram_tensor("outT2", [C, T], BF16, kind="ExternalOutput").ap()

    with tile.TileContext(nc, trace_sim=trace_sim) as tc:
        with tc.tile_pool(name="persist", bufs=1) as pp, \
             tc.tile_pool(name="ps_s", bufs=3, space="PSUM") as ps_s, \
             tc.tile_pool(name="ps_t", bufs=2, space="PSUM") as ps_t:

            # ---- PE warmup: ramp the tensor-engine P-state during the
            # input DMA window with dummy back-to-back matmuls ----
            wa_t = pp.tile([128, 128], BF16, tag="wa")
            nc.vector.memset(wa_t[:], 0.0)
            for _ in range(16):
                psw = ps_s.tile([128, 1024], F32, tag="s")
                nc.tensor.matmul(psw[:, 0:128], wa_t[:], wa_t[:],
                                 start=True, stop=True)

            # ---- persistent small tiles (merged blobs) ----
            cstf = pp.tile([128, 136], F32, tag="cstf")
            cstb = pp.tile([128, 256], BF16, tag="cstb")
            gg_t = cstf[:, 0:128]
            gw2_t = cstf[:, 128:129]
            gb2_t = cstf[:, 129:130]
            icon0 = cstf.bitcast(mybir.dt.uint32)[:, 130:131]
            icon1 = cstf.bitcast(mybir.dt.uint32)[:, 131:132]
            mask_t = cstb[:, 0:128]
            ident_t = cstb[:, 128:256]

            def lamn_sl(j):
                return cstf[:, 132 + j:133 + j]
            c15_t = pp.tile([128, 1], F32, tag="c15")
            nc.vector.memset(c15_t[:], 1.5)

            # ---- persistent activation tensors ----
            q1t = [pp.tile([128, T], BF16, tag=f"q1t{i}", name=f"q1t{i}") for i in range(2)]
            q2t = [pp.tile([128, T], BF16, tag=f"q2t{i}", name=f"q2t{i}") for i in range(2)]
            k1t = [pp.tile([128, T], BF16, tag=f"k1t{i}", name=f"k1t{i}") for i in range(2)]
            k2t = [pp.tile([128, T], BF16, tag=f"k2t{i}", name=f"k2t{i}") for i in range(2)]
            vt = [pp.tile([128, NH * D], BF16, tag=f"vt{i}", name=f"vt{i}") for i in range(NT)]
            # yT per head [D=128, T] bf16 (post-groupnorm)
            yt = [pp.tile([128, T], BF16, tag=f"yt{j}", name=f"yt{j}") for j in range(NH)]
            # wcT: [512, C] as 4 f-chunk tiles (one per head)
            wct = [pp.tile([128, C], BF16, tag=f"wct{j}", name=f"wct{j}") for j in range(NH)]

            # attention working pool opened BEFORE the loads pool so the
            # loads pool can be released first (pools close LIFO)
            wp_cm = tc.tile_pool(name="aw", bufs=2)
            wp = wp_cm.__enter__()

            # ================= q/k projection loads =================
            lp_cm = tc.tile_pool(name="loads", bufs=1)
            lp = lp_cm.__enter__()
            xt = [lp.tile([128, T], BF16, tag=f"xt{i}", name=f"xt{i}") for i in range(8)]
            wqm = {}
            for nm in ("q1", "q2", "k1", "k2"):
                wqm[nm] = [lp.tile([128, 4 * NH * HS], BF16,
                                   tag=f"w{nm}{h}", name=f"w{nm}{h}")
                           for h in range(2)]
            wvm = [lp.tile([128, 4 * NH * D], BF16, tag=f"wvm{h}",
                           name=f"wvm{h}") for h in range(2)]

            def wq_sl(nm, cc, oc):
                return wqm[nm][cc // 4][:, (cc % 4) * 256 + oc * 128:
                                        (cc % 4) * 256 + oc * 128 + 128]

            def wv_sl(cc):
                return wvm[cc // 4][:, (cc % 4) * 512:(cc % 4) * 512 + 512]

            # ---- DMA schedule (3 issue queues: SP/ACT/Pool): x chunks 0-3
            # and the h=0 halves of q1/k1/q2/k2 first (the first 4 cc-chunks
            # of each projection can start on those), then x4-7 + h=1 ----
            wdsc = {"q1": wq1_d, "k1": wk1_d, "q2": wq2_d, "k2": wk2_d}

            def w_dma(eng, nm, h):
                eng.dma_start(wqm[nm][h][:],
                              wdsc[nm][h * 128:(h + 1) * 128, :])

            def x_dma(eng, i):
                eng.dma_start(xt[i][:], xT_d[i * 128:(i + 1) * 128, :])

            # weights are small (256KB per half) and are needed by every
            # x-chunk matmul: land them first, then stream x in cc order
            w_dma(nc.sync, "q1", 0)
            w_dma(nc.scalar, "k1", 0)
            w_dma(nc.gpsimd, "q2", 0)
            w_dma(nc.sync, "q1", 1)
            w_dma(nc.scalar, "k1", 1)
            w_dma(nc.gpsimd, "k2", 0)
            x_dma(nc.sync, 0)
            x_dma(nc.scalar, 1)
            x_dma(nc.gpsimd, 2)
            x_dma(nc.sync, 3)
            x_dma(nc.scalar, 4)
            x_dma(nc.gpsimd, 5)
            x_dma(nc.sync, 6)
            x_dma(nc.scalar, 7)
            w_dma(nc.gpsimd, "q2", 1)
            w_dma(nc.gpsimd, "k2", 1)
            nc.gpsimd.dma_start(cstb[:], cstb_d)
            # wv needed by the first background v chunks (~35us in)
            nc.sync.dma_start(wvm[0][:], wv_d[0:128, :])
            nc.scalar.dma_start(wvm[1][:], wv_d[128:256, :])
            # wc + groupnorm consts needed late
            for j in range(NH):
                eng = (nc.sync, nc.scalar)[j % 2]
                eng.dma_start(wct[j][:], wc_d[j * 128:(j + 1) * 128, :])
            nc.gpsimd.dma_start(cstf[:], cstf_d)

            # qT/kT projections: out [o=128, t=512] = W^T_chunk.T @ xT
            _qk_ps = {}

            def emit_qk(nm, dst, oc, half):
                if half == 0:
                    _qk_ps[(nm, oc)] = (ps_s.tile([128, 1024], F32, tag="s", name="qkA"),
                                        ps_s.tile([128, 1024], F32, tag="s", name="qkB"))
                psA, psB = _qk_ps[(nm, oc)]
                for cc in range(4 * half, 4 * half + 4):
                    for tb in range(T // 512):
                        ph = (psA, psB)[tb // 2]
                        mi = nc.tensor.matmul(
                            ph[:, (tb % 2) * 512:(tb % 2) * 512 + 512],
                            wq_sl(nm, cc, oc),
                            xt[cc][:, tb * 512:(tb + 1) * 512],
                            start=(cc == 0), stop=(cc == 7),
                            skip_group_check=True)
                        if tb > 0:
                            mi.ins.ldweights = False
                if half == 1:
                    for tb2 in range(2):
                        src = (psA, psB)[tb2][:]
                        if nm in ("q1", "k1"):
                            nc.scalar.copy(
                                dst[oc][:, tb2 * 1024:(tb2 + 1) * 1024], src)
                        else:
                            nc.vector.tensor_copy(
                                dst[oc][:, tb2 * 1024:(tb2 + 1) * 1024], src)

            def emit_qk_full(nm, dst, oc):
                emit_qk(nm, dst, oc, 0)
                emit_qk(nm, dst, oc, 1)

            # oc=0 projections up front (heads 0-1 attention needs them)
            for nm, dst in (("q1", q1t), ("k1", k1t), ("q2", q2t), ("k2", k2t)):
                emit_qk_full(nm, dst, 0)

            # v projection chunks are emitted lazily
            v_done = [False] * NT

            def emit_v(tch):
                if v_done[tch]:
                    return
                v_done[tch] = True
                ps = ps_t.tile([128, NH * D], F32, tag="tz")
                for cc in range(8):
                    nc.tensor.matmul(
                        ps[:],
                        xt[cc][:, tch * 128:(tch + 1) * 128],
                        wv_sl(cc),
                        start=(cc == 0), stop=(cc == 7))
                if tch % 2 == 0:
                    nc.vector.tensor_copy(vt[tch][:], ps[:])
                else:
                    nc.scalar.copy(vt[tch][:], ps[:])

            # background PE work drained between attention phases of
            # heads 0-1: v chunks (just-in-time via the phase2 safety),
            # then the oc=1 projections (needed only by pair (2,3))
            bg = [(lambda t=t: emit_v(t)) for t in range(NT)]
            for nm, dst in (("q1", q1t), ("k1", k1t),
                            ("q2", q2t), ("k2", k2t)):
                bg.append(lambda nm=nm, dst=dst: emit_qk_full(nm, dst, 1))

            def bg_drain(n):
                for _ in range(min(n, len(bg))):
                    bg.pop(0)()

            # ================= attention per head =================
            head_ctx = {}

            def phase1_qt(j, qb, qq, AB):
                oc, po = divmod(j * HS, 128)
                qt = qb * 4 + qq
                nk = qt + 1
                nkb2 = (nk + 7) // 8   # 1024-wide S psum tiles
                e1 = wp.tile([128, T], BF16, tag="e1", name="e1", bufs=4)
                e2 = wp.tile([128, T], BF16, tag="e2", name="e2", bufs=3)
                dd = wp.tile([128, 2], F32, tag="dd", name="dd", bufs=4)
                d1c = wp.tile([128, 2], F32, tag="d1c", name="d1c", bufs=4)
                d2c = wp.tile([128, 2], F32, tag="d2c", name="d2c", bufs=4)
                for mi, (qsrc, ksrc, erow, dcol) in enumerate(
                        ((q1t, k1t, e1, d1c), (q2t, k2t, e2, d2c))):
                    for kb in range(nkb2):
                        w = min(1024, nk * 128 - kb * 1024)
                        ps = ps_s.tile([128, 1024], F32, tag="s",
                                       name="ps")
                        off = qt * 128 - kb * 1024  # diag block col
                        for hf in range(2):
                            wh = min(512, w - hf * 512)
                            if wh <= 0:
                                break
                            diag_here = (kb == nkb2 - 1 and
                                         hf * 512 <= off < hf * 512 + wh)
                            mm = nc.tensor.matmul(
                                ps[:, hf * 512:hf * 512 + wh],
                                qsrc[oc][po:po + HS,
                                         qt * 128:(qt + 1) * 128],
                                ksrc[oc][po:po + HS,
                                         kb * 1024 + hf * 512:
                                         kb * 1024 + hf * 512 + wh],
                                start=True, stop=not diag_here,
                                skip_group_check=diag_here)
                            if kb + hf > 0:
                                mm.ins.ldweights = False
                        if kb == nkb2 - 1:
                            # mask diagonal 128-block on PE
                            nc.tensor.matmul(
                                ps[:, off:off + 128],
                                ident_t, mask_t,
                                start=False, stop=True,
                                skip_group_check=True)
                        # accum straight into dd when a single psum covers
                        # the row (saves two DVE copies per qt)
                        acc = (dd[:, mi:mi + 1] if nkb2 == 1
                               else dcol[:, kb:kb + 1])
                        nc.scalar.activation(
                            erow[:, kb * 1024:kb * 1024 + w],
                            ps[:, :w], AF.Exp, scale=SCALE,
                            accum_out=acc)
                # denominators -> rr = [1/D1, 1/D2]
                rr = wp.tile([128, 2], F32, tag="rr", name="rr", bufs=4)
                if nkb2 > 1:
                    nc.vector.tensor_reduce(
                        dd[:, 0:1], d1c[:, 0:nkb2],
                        axis=mybir.AxisListType.X, op=ALU.add)
                    nc.vector.tensor_reduce(
                        dd[:, 1:2], d2c[:, 0:nkb2],
                        axis=mybir.AxisListType.X, op=ALU.add)
                nc.vector.reciprocal(rr[:], dd[:, 0:2])
                # sc2 = -lam / D2 (per-partition scalar)
                sc2 = wp.tile([128, 1], F32, tag="sc2", name="sc2", bufs=4)
                nc.vector.tensor_tensor(sc2[:], rr[:, 1:2],
                                        lamn_sl(j), ALU.mult)
                # att = e1*r1 + e2*sc2 (fully normalized diff-attention
                # row): tensor_scalar (4x) + scalar_tensor_tensor
                etmp = wp.tile([128, T], BF16, tag="etmp", name="etmp",
                               bufs=1)
                nc.vector.tensor_scalar(
                    etmp[:, :nk * 128], e2[:, :nk * 128], sc2[:, 0:1],
                    None, op0=ALU.mult)
                nc.vector.scalar_tensor_tensor(
                    e1[:, :nk * 128], e1[:, :nk * 128], rr[:, 0:1],
                    etmp[:, :nk * 128], op0=ALU.mult, op1=ALU.add)
                # transpose this attention row on the DMA xbar:
                # [q=128, nk*128] -> nk transposed blocks [k=128, 128]
                nc.sync.dma_start_transpose(
                    AB[:, 0:nk, qq, :], e1[:, :nk * 128])

            def phase1_pair(jA, jB, qb):
                # qt-interleaved emission across the head pair: the PE always
                # has the other head's independent S-chunk while ACT/DVE
                # drain this one's exp/combine chain
                ABs = {}
                for j in (jA, jB):
                    # AB layout [k=128, kc, qq, q-col]: z-matmul rhs slices
                    # contiguous (strided moving operands are silently wrong
                    # on HW; strided DMA-transpose OUT is fine)
                    ABs[j] = wp.tile([128, NT, 4, 128], BF16, tag="AB",
                                     name="AB", bufs=2)
                for qq in range(4):
                    for j in (jA, jB):
                        phase1_qt(j, qb, qq, ABs[j])
                return ABs

            def phase2(j, qb, AB):
                ytr, s1p, s2p = head_ctx[j]
                nkc = qb * 4 + 4
                # yT[d, qblk] = sum_kc v_kc.T @ attT_kc   (N=512)
                py = ps_t.tile([128, 512], F32, tag="tz", name="py")
                for kc in range(nkc):
                    emit_v(kc)
                    qq0 = max(0, kc - qb * 4)
                    zw = qq0 * 128
                    nc.tensor.matmul(
                        py[:, zw:],
                        vt[kc][:, j * 128:(j + 1) * 128],
                        AB[:, kc, qq0:4, :].rearrange("p q c -> p (q c)"),
                        start=(kc == 0), stop=(kc == nkc - 1),
                        skip_group_check=True)
                # copy to ytr with fused stats accumulation on DVE (no
                # 183ns ACT accumulator-read tax; ACT is the busier engine)
                nc.vector.tensor_scalar(
                    ytr[:, qb * 512:(qb + 1) * 512], py[:], 1.0, 0.0,
                    op0=ALU.mult, op1=ALU.add,
                    accum_out=s1p[:, qb:qb + 1])
                ysq = wp.tile([128, 512], BF16, tag="ysq", name="ysq",
                              bufs=1)
                ysrc = ytr[:, qb * 512:(qb + 1) * 512]
                nc.vector.scalar_tensor_tensor(
                    ysq[:], ysrc, 1.0, ysrc,
                    op0=ALU.mult, op1=ALU.mult,
                    accum_out=s2p[:, qb:qb + 1])

            def gn_final(j):
                ytr, s1p, s2p = head_ctx[j]
                s12 = wp.tile([128, 2], F32, tag="s12", name="s12")
                nc.vector.tensor_reduce(s12[:, 0:1], s1p[:, 0:NT // 4],
                                        axis=mybir.AxisListType.X, op=ALU.add)
                nc.vector.tensor_reduce(s12[:, 1:2], s2p[:, 0:NT // 4],
                                        axis=mybir.AxisListType.X, op=ALU.add)
                pg = ps_t.tile([128, 2], F32, tag="tz", name="pg")
                nc.tensor.matmul(pg[:], gg_t, s12[:], start=True, stop=True)
                # mneg = -mean; nvar = mean^2 - E[y^2] = -var
                mneg = wp.tile([128, 1], F32, tag="mneg", name="mneg")
                nc.scalar.mul(mneg[:], pg[:, 0:1], -1.0 / (T * 4))
                msq = wp.tile([128, 1], F32, tag="msq")
                nc.scalar.mul(msq[:], pg[:, 1:2], 1.0 / (T * 4))
                nvar = wp.tile([128, 1], F32, tag="nvar")
                nc.vector.scalar_tensor_tensor(
                    nvar[:], mneg[:], mneg[:, 0:1], msq[:],
                    op0=ALU.mult, op1=ALU.subtract)
                vpe = wp.tile([128, 1], F32, tag="vpe")
                nc.vector.tensor_scalar(vpe[:], nvar[:], -1.0, EPS,
                                        op0=ALU.mult, op1=ALU.add)  # var+eps
                # rsqrt(var+eps) on DVE only: quake seed + Newton iters
                rstd = wp.tile([128, 1], F32, tag="rstd")
                yi = wp.tile([128, 1], F32, tag="yi")
                nc.vector.tensor_tensor(yi.bitcast(mybir.dt.uint32)[:],
                                        vpe.bitcast(mybir.dt.uint32)[:],
                                        icon0,
                                        ALU.logical_shift_right)
                nc.vector.tensor_tensor(yi.bitcast(mybir.dt.uint32)[:],
                                        icon1,
                                        yi.bitcast(mybir.dt.uint32)[:],
                                        ALU.subtract)
                vneg = wp.tile([128, 1], F32, tag="vneg")
                nc.vector.tensor_scalar_mul(vneg[:], vpe[:], -0.5)
                ytmp = wp.tile([128, 1], F32, tag="ytmp")
                for _ in range(2):
                    nc.vector.tensor_tensor(ytmp[:], yi[:], yi[:], ALU.mult)
                    nc.vector.scalar_tensor_tensor(
                        ytmp[:], ytmp[:], vneg[:, 0:1], c15_t[:],
                        op0=ALU.mult, op1=ALU.add)  # 1.5 - 0.5 v y^2
                    nc.vector.tensor_tensor(yi[:], yi[:], ytmp[:], ALU.mult)
                nc.vector.tensor_copy(rstd[:], yi[:])
                aff_a = wp.tile([128, 1], F32, tag="aff_a")
                nc.vector.tensor_tensor(aff_a[:], rstd[:], gw2_t, ALU.mult)
                aff_b = wp.tile([128, 1], F32, tag="aff_b")
                nc.vector.scalar_tensor_tensor(
                    aff_b[:], mneg[:], aff_a[:, 0:1], gb2_t,
                    op0=ALU.mult, op1=ALU.add)  # gb2 - mean*aff_a
                # affine on DVE (4x): yt = ytr*aff_a + aff_b
                nc.vector.tensor_scalar(yt[j][:], ytr[:], aff_a[:, 0:1],
                                        aff_b[:, 0:1],
                                        op0=ALU.mult, op1=ALU.add)

            def new_head(j):
                head_ctx[j] = (
                    wp.tile([128, T], BF16, tag="ytr", name="ytr", bufs=3),
                    wp.tile([128, 4], F32, tag="s1p", name="s1p"),
                    wp.tile([128, 4], F32, tag="s2p", name="s2p"))

            # qb order (1,2,3,0): the serial end-of-pair chain (last exp ->
            # combine -> transpose -> z -> stats -> gn) runs on the smallest
            # q-block, shrinking the pair-boundary latency
            QBS = (1, 2, 3, 0)

            # ---- pair (0,1): bg (v + oc1 projections) fills the PE ----
            for j in (0, 1):
                new_head(j)
            for qb in QBS:
                ABs = phase1_pair(0, 1, qb)
                bg_drain(5)
                phase2(0, qb, ABs[0])
                if qb == 0:
                    bg_drain(len(bg))
                    gn_final(0)
                phase2(1, qb, ABs[1])
                if qb == 0:
                    gn_final(1)

            # pair boundary: release x/weight tiles, open the partial pool
            lp_cm.__exit__(None, None, None)
            dp_cm = tc.tile_pool(name="drain", bufs=1)
            dp = dp_cm.__enter__()
            p01 = [dp.tile([128, T], BF16, tag=f"p01_{ocb}",
                           name=f"p01_{ocb}") for ocb in range(8)]

            # out-proj j=0,1 partials: fill PE during the ACT-bound
            # (2,3) pair. Two ocbs per qb slot.
            fill_q = list(range(8))

            def emit_fill(ocb):
                for tb in range(T // 512):
                    pt = ps_t.tile([128, 512], F32, tag="tz", name="fl")
                    for j in (0, 1):
                        nc.tensor.matmul(
                            pt[:],
                            wct[j][:, ocb * 128:(ocb + 1) * 128],
                            yt[j][:, tb * 512:(tb + 1) * 512],
                            start=(j == 0), stop=(j == 1),
                            skip_group_check=True)
                    nc.vector.tensor_copy(
                        p01[ocb][:, tb * 512:(tb + 1) * 512], pt[:])
                # ship the j01 partial to the host during the attention
                # phase (DMA engines are idle here); host adds the partials
                nc.gpsimd.dma_start(
                    outT2_d[ocb * 128:(ocb + 1) * 128, :], p01[ocb][:])

            # ---- pair (2,3): fills + attention ----
            for j in (2, 3):
                new_head(j)
            for qb in QBS:
                ABs = phase1_pair(2, 3, qb)
                if qb != 0:
                    for _ in range(2):
                        if fill_q:
                            emit_fill(fill_q.pop(0))
                phase2(2, qb, ABs[2])
                if qb == 0:
                    # the held-back fills give the PE gn-independent work
                    # while the gn chains run
                    gn_final(2)
                    if fill_q:
                        emit_fill(fill_q.pop(0))
                phase2(3, qb, ABs[3])
                if qb == 0:
                    gn_final(3)
                    while fill_q:
                        emit_fill(fill_q.pop(0))

            # ================= output projection =================
            # per ocb: ident-add of the j01 partial, then j=2, then j=3.
            # gn(3) is emitted just before ocb0 so its ACT->DVE chain hides
            # behind the gn-independent ident-adds + j=2 matmuls.
            def s_halves():
                psA = ps_s.tile([128, 1024], F32, tag="s", name="psA")
                psB = ps_s.tile([128, 1024], F32, tag="s", name="psB")
                return (psA, psB)

            def op_mms23(ocb, halves):
                for j in (2, 3):
                    for tb in range(T // 512):
                        ph = halves[tb // 2]
                        mi = nc.tensor.matmul(
                            ph[:, (tb % 2) * 512:(tb % 2) * 512 + 512],
                            wct[j][:, ocb * 128:(ocb + 1) * 128],
                            yt[j][:, tb * 512:(tb + 1) * 512],
                            start=(j == 2), stop=(j == 3),
                            skip_group_check=True)
                        if tb > 0:
                            mi.ins.ldweights = False

            _fin_rr = [0]

            def op_fin(ocb, halves):
                for tb2 in range(2):
                    ob = dp.tile([128, 1024], BF16, tag="ob", bufs=4,
                                 name="ob")
                    r = _fin_rr[0] = (_fin_rr[0] + 1) % 2
                    if r == 0:
                        nc.vector.tensor_copy(ob[:], halves[tb2][:])
                    else:
                        nc.scalar.copy(ob[:], halves[tb2][:])
                    eng = (nc.sync, nc.gpsimd, nc.scalar)[(2 * ocb + tb2) % 3]
                    eng.dma_start(
                        outT_d[ocb * 128:(ocb + 1) * 128,
                               tb2 * 1024:(tb2 + 1) * 1024], ob[:])

            for ocb in range(8):
                halves = s_halves()
                op_mms23(ocb, halves)
                op_fin(ocb, halves)
            dp_cm.__exit__(None, None, None)
            wp_cm.__exit__(None, None, None)

    nc.compile()
    return nc


def _prep_inputs(inputs):
    bf = ml_dtypes.bfloat16
    x = np.asarray(inputs["x"], np.float32)
    Wq1 = np.asarray(inputs["Wq1"], np.float32)
    Wq2 = np.asarray(inputs["Wq2"], np.float32)
    Wk1 = np.asarray(inputs["Wk1"], np.float32)
    Wk2 = np.asarray(inputs["Wk2"], np.float32)
    Wv = np.asarray(inputs["Wv"], np.float32)
    Wc = np.asarray(inputs["Wc"], np.float32)
    gn_w = np.asarray(inputs["gn_w"], np.float32)
    gn_b = np.asarray(inputs["gn_b"], np.float32)
    gamma = np.asarray(inputs["gamma"], np.float32)

    def sig(v):
        return 1.0 / (1.0 + np.exp(-v))

    lam = (sig(np.asarray(inputs["lq1"], np.float32).reshape(H)
               * np.asarray(inputs["lk1"], np.float32).reshape(H))
           - sig(np.asarray(inputs["lq2"], np.float32).reshape(H)
                 * np.asarray(inputs["lk2"], np.float32).reshape(H))
           + LAMBDA_INIT)

    mask = np.where(np.arange(128)[None, :] <= np.arange(128)[:, None],
                    0.0, NEG).astype(bf)
    ident = np.eye(128, dtype=np.float32).astype(bf)
    gg = (np.arange(128)[:, None] // 4 == np.arange(128)[None, :] // 4
          ).astype(np.float32)
    c1 = 1.0 - LAMBDA_INIT
    gw2 = (gn_w * gamma * c1).astype(np.float32).reshape(128, 1)
    gb2 = (gn_b * gamma * c1).astype(np.float32).reshape(128, 1)

    icon = np.zeros((128, 2), np.uint32)
    icon[:, 0] = 1
    icon[:, 1] = 0x5f375a00
    cstb = np.concatenate([mask, ident], axis=1)
    xTb = [np.ascontiguousarray(x[b].T).astype(bf) for b in range(B)]
    in_maps = []
    for core in range(N_CORES):
        b, hg = divmod(core, N_CORES // B)
        qs = hg * NH * HS          # 256-wide q/k slice
        vs = hg * NH * D           # 512-wide v / y2 slice
        lamn = np.repeat(-lam[hg * NH:(hg + 1) * NH].reshape(1, NH),
                         128, axis=0).astype(np.float32)
        cstf = np.concatenate(
            [gg, gw2, gb2, icon.view(np.float32), lamn],
            axis=1).astype(np.float32)
        def mtiles(wt, cols):
            # [1024, cols] -> merged 2-tile layout [256, 4*cols]
            return np.ascontiguousarray(
                wt.reshape(2, 4, 128, cols).transpose(0, 2, 1, 3)
                .reshape(256, 4 * cols)).astype(bf)

        in_maps.append({
            "xT": xTb[b],
            "wq1T": mtiles(Wq1[qs:qs + NH * HS, :].T, NH * HS),
            "wq2T": mtiles(Wq2[qs:qs + NH * HS, :].T, NH * HS),
            "wk1T": mtiles(Wk1[qs:qs + NH * HS, :].T, NH * HS),
            "wk2T": mtiles(Wk2[qs:qs + NH * HS, :].T, NH * HS),
            "wvT": mtiles(Wv[vs:vs + NH * D, :].T, NH * D),
            "wcT": np.ascontiguousarray(Wc[:, vs:vs + NH * D].T).astype(bf),
            "cstf": cstf,
            "cstb": cstb,
        })
    return in_maps


def kernel(**inputs):
    if "nc" not in _cache:
        _cache["nc"] = _build()
    nc = _cache["nc"]
    in_maps = _prep_inputs(inputs)
    res = bass_utils.run_bass_kernel_spmd(
        nc, in_maps, core_ids=list(range(N_CORES)),
        **_cache.get("run_kwargs", {}))
    _cache["last_result"] = res
    out = np.zeros((B, T, C), np.float32)
    for core in range(N_CORES):
        b = core // (N_CORES // B)
        out[b] += res.results[core]["outT"].T.astype(np.float32)
        out[b] += res.results[core]["outT2"].T.astype(np.float32)
    return out


# revision 60
# speedup vs baseline: 1.0531x; 1.0077x over previous
"""Trainium2 Bass kernel for nn_MultiHeadDiffAttention (B=2,T=2048,C=1024,H=16).

Sharding: 8 cores = data-parallel over B(2) x tensor-parallel over 4 head-groups
(4 heads each). Each core computes q/k/v projections for its heads, causal
differential attention, per-head GroupNorm, and a partial output projection
(its 512 columns of y2 against Wc). Host sums the 4 partials per batch.

v8 design (major wins over the v2 baseline, 434us -> ~365us):
  - attention-row transpose moved off the PE onto the DMA xbar
    (dma_start_transpose): saves ~70K PE columns + 544 LDWEIGHTS and all
    psum->sbuf attT casts (~90us of ACT/DVE). The combine then produces
    fully normalized rows att = e1/D1 - lam*e2/D2 via one DVE
    tensor_scalar (4x mode) + one scalar_tensor_tensor. NOTE: the z-matmul
    rhs must be CONTIGUOUS (AB layout [k, kc, qq, col]) - strided 3D
    moving operands are silently wrong on real HW (sim disagrees).
  - phase1 emits the two heads of a pair interleaved at q-tile granularity
    so the PE always holds independent S-work while ACT/DVE drain a chain.
  - qb order (1,2,3,0): the serial pair-end chain (exp->combine->
    transpose->z->stats->groupnorm) runs on the smallest q-block.
  - per-queue DMA order matches consumption (w-h0 halves + x0-3, then h1 +
    x4-7); merged constant blobs cut issue count (each dma_start costs
    ~600ns of queue time).
  - head-0/1 output-projection partials are computed during the ACT-bound
    pair-(2,3) phase and shipped to the host as a second bf16 output
    (outT2) on the idle DMA engines; the host adds the partials. The final
    device phase is only the j=2,3 projection.
  - groupnorm stats accumulate on DVE (tensor_scalar accum_out; no 183ns
    ACT accumulator-read tax), rsqrt via quake seed + 2 Newton iters.
  - outT/outT2 DMA'd in bf16 (halves output traffic); host upcasts.
"""

import sys

for _p in ("/opt/trn_rl_repo", "/root/.axon_site/_ro/trn_rl_repo"):
    if _p not in sys.path:
        sys.path.insert(0, _p)

import math
import numpy as np
import ml_dtypes

import concourse.bass as bass
import concourse.bacc as bacc
import concourse.tile as tile
import concourse.mybir as mybir
from concourse import bass_utils

F32 = mybir.dt.float32
BF16 = mybir.dt.bfloat16
AF = mybir.ActivationFunctionType
ALU = mybir.AluOpType

B, T, C = 2, 2048, 1024
H = 16
HS = C // H           # 64
D = 2 * HS            # 128 v-channels per head
NH = 4                # heads per core
N_CORES = 8
NT = T // 128         # 16 q-tiles
LAMBDA_INIT = 0.8 - 0.6 * math.exp(-0.3 * (12 - 1))
EPS = 1e-5
SCALE = 1.0 / math.sqrt(HS)
NEG = -30000.0

_cache = {}


def _build(T=T, trace_sim=False, nh=NH):
    NT = T // 128
    nc = bacc.Bacc("TRN2", target_bir_lowering=False, debug=False,
                   num_devices=N_CORES)

    def din(name, shape, dt=BF16):
        return nc.dram_tensor(name, shape, dt, kind="ExternalInput").ap()

    xT_d = din("xT", [C, T])
    # q/k/v weights arrive in host-merged tile layout: [2 tiles x 128
    # partitions, 4 chunks x cols] so each needs only a few DMAs
    wq1_d = din("wq1T", [256, 4 * NH * HS])
    wq2_d = din("wq2T", [256, 4 * NH * HS])
    wk1_d = din("wk1T", [256, 4 * NH * HS])
    wk2_d = din("wk2T", [256, 4 * NH * HS])
    wv_d = din("wvT", [256, 4 * NH * D])
    wc_d = din("wcT", [NH * D, C])
    # merged constants: one f32 blob (gg | gw2 | gb2 | icon-bits | lamn) and
    # one bf16 blob (mask | ident) -> 2 DMA issues instead of 7
    cstf_d = din("cstf", [128, 136], F32)
    cstb_d = din("cstb", [128, 256])
    outT_d = nc.dram_tensor("outT", [C, T], BF16, kind="ExternalOutput").ap()
    outT2_d = nc.dram_tensor("outT2", [C, T], BF16, kind="ExternalOutput").ap()

    with tile.TileContext(nc, trace_sim=trace_sim) as tc:
        with tc.tile_pool(name="persist", bufs=1) as pp, \
             tc.tile_pool(name="ps_s", bufs=3, space="PSUM") as ps_s, \
             tc.tile_pool(name="ps_t", bufs=2, space="PSUM") as ps_t:

            # ---- PE warmup: ramp the tensor-engine P-state during the
            # input DMA window with dummy back-to-back matmuls ----
            wa_t = pp.tile([128, 128], BF16, tag="wa")
            nc.vector.memset(wa_t[:], 0.0)
            for _ in range(16):
                psw = ps_s.tile([128, 1024], F32, tag="s")
                nc.tensor.matmul(psw[:, 0:128], wa_t[:], wa_t[:],
                                 start=True, stop=True)

            # ---- persistent small tiles (merged blobs) ----
            cstf = pp.tile([128, 136], F32, tag="cstf")
            cstb = pp.tile([128, 256], BF16, tag="cstb")
            gg_t = cstf[:, 0:128]
            gw2_t = cstf[:, 128:129]
            gb2_t = cstf[:, 129:130]
            icon0 = cstf.bitcast(mybir.dt.uint32)[:, 130:131]
            icon1 = cstf.bitcast(mybir.dt.uint32)[:, 131:132]
            mask_t = cstb[:, 0:128]
            ident_t = cstb[:, 128:256]

            def lamn_sl(j):
                return cstf[:, 132 + j:133 + j]
            c15_t = pp.tile([128, 1], F32, tag="c15")
            nc.vector.memset(c15_t[:], 1.5)

            # ---- persistent activation tensors ----
            q1t = [pp.tile([128, T], BF16, tag=f"q1t{i}", name=f"q1t{i}") for i in range(2)]
            q2t = [pp.tile([128, T], BF16, tag=f"q2t{i}", name=f"q2t{i}") for i in range(2)]
            k1t = [pp.tile([128, T], BF16, tag=f"k1t{i}", name=f"k1t{i}") for i in range(2)]
            k2t = [pp.tile([128, T], BF16, tag=f"k2t{i}", name=f"k2t{i}") for i in range(2)]
            vt = [pp.tile([128, NH * D], BF16, tag=f"vt{i}", name=f"vt{i}") for i in range(NT)]
            # yT per head [D=128, T] bf16 (post-groupnorm)
            yt = [pp.tile([128, T], BF16, tag=f"yt{j}", name=f"yt{j}") for j in range(NH)]
            # wcT: [512, C] as 4 f-chunk tiles (one per head)
            wct = [pp.tile([128, C], BF16, tag=f"wct{j}", name=f"wct{j}") for j in range(NH)]

            # attention working pool opened BEFORE the loads pool so the
            # loads pool can be released first (pools close LIFO)
            wp_cm = tc.tile_pool(name="aw", bufs=2)
            wp = wp_cm.__enter__()

            # ================= q/k projection loads =================
            lp_cm = tc.tile_pool(name="loads", bufs=1)
            lp = lp_cm.__enter__()
            xt = [lp.tile([128, T], BF16, tag=f"xt{i}", name=f"xt{i}") for i in range(8)]
            wqm = {}
            for nm in ("q1", "q2", "k1", "k2"):
                wqm[nm] = [lp.tile([128, 4 * NH * HS], BF16,
                                   tag=f"w{nm}{h}", name=f"w{nm}{h}")
                           for h in range(2)]
            wvm = [lp.tile([128, 4 * NH * D], BF16, tag=f"wvm{h}",
                           name=f"wvm{h}") for h in range(2)]

            def wq_sl(nm, cc, oc):
                return wqm[nm][cc // 4][:, (cc % 4) * 256 + oc * 128:
                                        (cc % 4) * 256 + oc * 128 + 128]

            def wv_sl(cc):
                return wvm[cc // 4][:, (cc % 4) * 512:(cc % 4) * 512 + 512]

            # ---- DMA schedule (3 issue queues: SP/ACT/Pool): x chunks 0-3
            # and the h=0 halves of q1/k1/q2/k2 first (the first 4 cc-chunks
            # of each projection can start on those), then x4-7 + h=1 ----
            wdsc = {"q1": wq1_d, "k1": wk1_d, "q2": wq2_d, "k2": wk2_d}

            def w_dma(eng, nm, h):
                eng.dma_start(wqm[nm][h][:],
                              wdsc[nm][h * 128:(h + 1) * 128, :])

            def x_dma(eng, i):
                eng.dma_start(xt[i][:], xT_d[i * 128:(i + 1) * 128, :])

            # per-queue arrival order matches the projections' consumption
            # order (h0 halves + x0-3, then h1 halves + x4-7) so the PE can
            # chase the DMA stream chunk by chunk
            w_dma(nc.sync, "q1", 0)
            w_dma(nc.scalar, "k1", 0)
            w_dma(nc.gpsimd, "q2", 0)
            x_dma(nc.sync, 0)
            x_dma(nc.scalar, 1)
            x_dma(nc.gpsimd, 2)
            x_dma(nc.sync, 3)
            w_dma(nc.scalar, "k1", 1)
            w_dma(nc.gpsimd, "k2", 0)
            w_dma(nc.sync, "q1", 1)
            x_dma(nc.scalar, 4)
            x_dma(nc.gpsimd, 5)
            x_dma(nc.sync, 6)
            x_dma(nc.scalar, 7)
            w_dma(nc.gpsimd, "q2", 1)
            w_dma(nc.gpsimd, "k2", 1)
            nc.gpsimd.dma_start(cstb[:], cstb_d)
            # wv needed by the first background v chunks (~35us in)
            nc.sync.dma_start(wvm[0][:], wv_d[0:128, :])
            nc.scalar.dma_start(wvm[1][:], wv_d[128:256, :])
            # wc + groupnorm consts needed late
            for j in range(NH):
                eng = (nc.sync, nc.scalar)[j % 2]
                eng.dma_start(wct[j][:], wc_d[j * 128:(j + 1) * 128, :])
            nc.gpsimd.dma_start(cstf[:], cstf_d)

            # qT/kT projections: out [o=128, t=512] = W^T_chunk.T @ xT
            _qk_ps = {}

            def emit_qk(nm, dst, oc, half):
                if half == 0:
                    _qk_ps[(nm, oc)] = (ps_s.tile([128, 1024], F32, tag="s", name="qkA"),
                                        ps_s.tile([128, 1024], F32, tag="s", name="qkB"))
                psA, psB = _qk_ps[(nm, oc)]
                for cc in range(4 * half, 4 * half + 4):
                    for tb in range(T // 512):
                        ph = (psA, psB)[tb // 2]
                        mi = nc.tensor.matmul(
                            ph[:, (tb % 2) * 512:(tb % 2) * 512 + 512],
                            wq_sl(nm, cc, oc),
                            xt[cc][:, tb * 512:(tb + 1) * 512],
                            start=(cc == 0), stop=(cc == 7),
                            skip_group_check=True)
                        if tb > 0:
                            mi.ins.ldweights = False
                if half == 1:
                    for tb2 in range(2):
                        src = (psA, psB)[tb2][:]
                        if nm in ("q1", "k1"):
                            nc.scalar.copy(
                                dst[oc][:, tb2 * 1024:(tb2 + 1) * 1024], src)
                        else:
                            nc.vector.tensor_copy(
                                dst[oc][:, tb2 * 1024:(tb2 + 1) * 1024], src)

            def emit_qk_full(nm, dst, oc):
                emit_qk(nm, dst, oc, 0)
                emit_qk(nm, dst, oc, 1)

            # oc=0 projections up front (heads 0-1 attention needs them)
            for nm, dst in (("q1", q1t), ("k1", k1t), ("q2", q2t), ("k2", k2t)):
                emit_qk_full(nm, dst, 0)

            # v projection chunks are emitted lazily
            v_done = [False] * NT

            def emit_v(tch):
                if v_done[tch]:
                    return
                v_done[tch] = True
                ps = ps_t.tile([128, NH * D], F32, tag="tz")
                for cc in range(8):
                    nc.tensor.matmul(
                        ps[:],
                        xt[cc][:, tch * 128:(tch + 1) * 128],
                        wv_sl(cc),
                        start=(cc == 0), stop=(cc == 7))
                if tch % 2 == 0:
                    nc.vector.tensor_copy(vt[tch][:], ps[:])
                else:
                    nc.scalar.copy(vt[tch][:], ps[:])

            # background PE work drained between attention phases of
            # heads 0-1: v chunks (just-in-time via the phase2 safety) with
            # the oc=1 projections interleaved so they don't clump at the
            # pair boundary where ACT/DVE would idle
            def _oc1(nm, dst):
                return lambda: emit_qk_full(nm, dst, 1)
            bg = ([(lambda t=t: emit_v(t)) for t in range(5)]
                  + [_oc1("q1", q1t)]
                  + [(lambda t=t: emit_v(t)) for t in range(5, 9)]
                  + [_oc1("k1", k1t)]
                  + [(lambda t=t: emit_v(t)) for t in range(9, 13)]
                  + [_oc1("q2", q2t)]
                  + [(lambda t=t: emit_v(t)) for t in range(13, 16)]
                  + [_oc1("k2", k2t)])

            def bg_drain(n):
                for _ in range(min(n, len(bg))):
                    bg.pop(0)()

            # ================= attention per head =================
            head_ctx = {}

            def phase1_qt(j, qb, qq, AB):
                oc, po = divmod(j * HS, 128)
                qt = qb * 4 + qq
                nk = qt + 1
                nkb2 = (nk + 7) // 8   # 1024-wide S psum tiles
                e1 = wp.tile([128, T], BF16, tag="e1", name="e1", bufs=4)
                e2 = wp.tile([128, T], BF16, tag="e2", name="e2", bufs=3)
                dd = wp.tile([128, 2], F32, tag="dd", name="dd", bufs=4)
                d1c = wp.tile([128, 2], F32, tag="d1c", name="d1c", bufs=4)
                d2c = wp.tile([128, 2], F32, tag="d2c", name="d2c", bufs=4)
                for mi, (qsrc, ksrc, erow, dcol) in enumerate(
                        ((q1t, k1t, e1, d1c), (q2t, k2t, e2, d2c))):
                    for kb in range(nkb2):
                        w = min(1024, nk * 128 - kb * 1024)
                        ps = ps_s.tile([128, 1024], F32, tag="s",
                                       name="ps")
                        off = qt * 128 - kb * 1024  # diag block col
                        for hf in range(2):
                            wh = min(512, w - hf * 512)
                            if wh <= 0:
                                break
                            diag_here = (kb == nkb2 - 1 and
                                         hf * 512 <= off < hf * 512 + wh)
                            mm = nc.tensor.matmul(
                                ps[:, hf * 512:hf * 512 + wh],
                                qsrc[oc][po:po + HS,
                                         qt * 128:(qt + 1) * 128],
                                ksrc[oc][po:po + HS,
                                         kb * 1024 + hf * 512:
                                         kb * 1024 + hf * 512 + wh],
                                start=True, stop=not diag_here,
                                skip_group_check=diag_here)
                            if kb + hf > 0:
                                mm.ins.ldweights = False
                        if kb == nkb2 - 1:
                            # mask diagonal 128-block on PE
                            nc.tensor.matmul(
                                ps[:, off:off + 128],
                                ident_t, mask_t,
                                start=False, stop=True,
                                skip_group_check=True)
                        # accum straight into dd when a single psum covers
                        # the row (saves two DVE copies per qt)
                        acc = (dd[:, mi:mi + 1] if nkb2 == 1
                               else dcol[:, kb:kb + 1])
                        nc.scalar.activation(
                            erow[:, kb * 1024:kb * 1024 + w],
                            ps[:, :w], AF.Exp, scale=SCALE,
                            accum_out=acc)
                # denominators -> rr = [1/D1, 1/D2]
                rr = wp.tile([128, 2], F32, tag="rr", name="rr", bufs=4)
                if nkb2 > 1:
                    nc.vector.tensor_reduce(
                        dd[:, 0:1], d1c[:, 0:nkb2],
                        axis=mybir.AxisListType.X, op=ALU.add)
                    nc.vector.tensor_reduce(
                        dd[:, 1:2], d2c[:, 0:nkb2],
                        axis=mybir.AxisListType.X, op=ALU.add)
                nc.vector.reciprocal(rr[:], dd[:, 0:2])
                # sc2 = -lam / D2 (per-partition scalar)
                sc2 = wp.tile([128, 1], F32, tag="sc2", name="sc2", bufs=4)
                nc.vector.tensor_tensor(sc2[:], rr[:, 1:2],
                                        lamn_sl(j), ALU.mult)
                # att = e1*r1 + e2*sc2 (fully normalized diff-attention
                # row): tensor_scalar (4x) + scalar_tensor_tensor
                etmp = wp.tile([128, T], BF16, tag="etmp", name="etmp",
                               bufs=1)
                nc.vector.tensor_scalar(
                    etmp[:, :nk * 128], e2[:, :nk * 128], sc2[:, 0:1],
                    None, op0=ALU.mult)
                nc.vector.scalar_tensor_tensor(
                    e1[:, :nk * 128], e1[:, :nk * 128], rr[:, 0:1],
                    etmp[:, :nk * 128], op0=ALU.mult, op1=ALU.add)
                # transpose this attention row on the DMA xbar:
                # [q=128, nk*128] -> nk transposed blocks [k=128, 128]
                nc.sync.dma_start_transpose(
                    AB[:, 0:nk, qq, :], e1[:, :nk * 128])

            def phase1_pair(jA, jB, qb):
                # qt-interleaved emission across the head pair: the PE always
                # has the other head's independent S-chunk while ACT/DVE
                # drain this one's exp/combine chain
                ABs = {}
                for j in (jA, jB):
                    # AB layout [k=128, kc, qq, q-col]: z-matmul rhs slices
                    # contiguous (strided moving operands are silently wrong
                    # on HW; strided DMA-transpose OUT is fine)
                    ABs[j] = wp.tile([128, NT, 4, 128], BF16, tag="AB",
                                     name="AB", bufs=2)
                for qq in range(4):
                    for j in (jA, jB):
                        phase1_qt(j, qb, qq, ABs[j])
                return ABs

            def phase2(j, qb, AB):
                ytr, s1p, s2p = head_ctx[j]
                nkc = qb * 4 + 4
                # yT[d, qblk] = sum_kc v_kc.T @ attT_kc   (N=512)
                py = ps_t.tile([128, 512], F32, tag="tz", name="py")
                for kc in range(nkc):
                    emit_v(kc)
                    qq0 = max(0, kc - qb * 4)
                    zw = qq0 * 128
                    nc.tensor.matmul(
                        py[:, zw:],
                        vt[kc][:, j * 128:(j + 1) * 128],
                        AB[:, kc, qq0:4, :].rearrange("p q c -> p (q c)"),
                        start=(kc == 0), stop=(kc == nkc - 1),
                        skip_group_check=True)
                # copy to ytr with fused stats accumulation on DVE (no
                # 183ns ACT accumulator-read tax; ACT is the busier engine)
                nc.vector.tensor_scalar(
                    ytr[:, qb * 512:(qb + 1) * 512], py[:], 1.0, 0.0,
                    op0=ALU.mult, op1=ALU.add,
                    accum_out=s1p[:, qb:qb + 1])
                ysq = wp.tile([128, 512], BF16, tag="ysq", name="ysq",
                              bufs=1)
                ysrc = ytr[:, qb * 512:(qb + 1) * 512]
                nc.vector.scalar_tensor_tensor(
                    ysq[:], ysrc, 1.0, ysrc,
                    op0=ALU.mult, op1=ALU.mult,
                    accum_out=s2p[:, qb:qb + 1])

            def gn_final(j):
                ytr, s1p, s2p = head_ctx[j]
                s12 = wp.tile([128, 2], F32, tag="s12", name="s12")
                nc.vector.tensor_reduce(s12[:, 0:1], s1p[:, 0:NT // 4],
                                        axis=mybir.AxisListType.X, op=ALU.add)
                nc.vector.tensor_reduce(s12[:, 1:2], s2p[:, 0:NT // 4],
                                        axis=mybir.AxisListType.X, op=ALU.add)
                pg = ps_t.tile([128, 2], F32, tag="tz", name="pg")
                nc.tensor.matmul(pg[:], gg_t, s12[:], start=True, stop=True)
                # mneg = -mean; nvar = mean^2 - E[y^2] = -var
                mneg = wp.tile([128, 1], F32, tag="mneg", name="mneg")
                nc.scalar.mul(mneg[:], pg[:, 0:1], -1.0 / (T * 4))
                msq = wp.tile([128, 1], F32, tag="msq")
                nc.scalar.mul(msq[:], pg[:, 1:2], 1.0 / (T * 4))
                nvar = wp.tile([128, 1], F32, tag="nvar")
                nc.vector.scalar_tensor_tensor(
                    nvar[:], mneg[:], mneg[:, 0:1], msq[:],
                    op0=ALU.mult, op1=ALU.subtract)
                vpe = wp.tile([128, 1], F32, tag="vpe")
                nc.vector.tensor_scalar(vpe[:], nvar[:], -1.0, EPS,
                                        op0=ALU.mult, op1=ALU.add)  # var+eps
                # rsqrt(var+eps) on DVE only: quake seed + Newton iters
                rstd = wp.tile([128, 1], F32, tag="rstd")
                yi = wp.tile([128, 1], F32, tag="yi")
                nc.vector.tensor_tensor(yi.bitcast(mybir.dt.uint32)[:],
                                        vpe.bitcast(mybir.dt.uint32)[:],
                                        icon0,
                                        ALU.logical_shift_right)
                nc.vector.tensor_tensor(yi.bitcast(mybir.dt.uint32)[:],
                                        icon1,
                                        yi.bitcast(mybir.dt.uint32)[:],
                                        ALU.subtract)
                vneg = wp.tile([128, 1], F32, tag="vneg")
                nc.vector.tensor_scalar_mul(vneg[:], vpe[:], -0.5)
                ytmp = wp.tile([128, 1], F32, tag="ytmp")
                for _ in range(2):
                    nc.vector.tensor_tensor(ytmp[:], yi[:], yi[:], ALU.mult)
                    nc.vector.scalar_tensor_tensor(
                        ytmp[:], ytmp[:], vneg[:, 0:1], c15_t[:],
                        op0=ALU.mult, op1=ALU.add)  # 1.5 - 0.5 v y^2
                    nc.vector.tensor_tensor(yi[:], yi[:], ytmp[:], ALU.mult)
                nc.vector.tensor_copy(rstd[:], yi[:])
                aff_a = wp.tile([128, 1], F32, tag="aff_a")
                nc.vector.tensor_tensor(aff_a[:], rstd[:], gw2_t, ALU.mult)
                aff_b = wp.tile([128, 1], F32, tag="aff_b")
                nc.vector.scalar_tensor_tensor(
                    aff_b[:], mneg[:], aff_a[:, 0:1], gb2_t,
                    op0=ALU.mult, op1=ALU.add)  # gb2 - mean*aff_a
                # affine on DVE (4x): yt = ytr*aff_a + aff_b
                nc.vector.tensor_scalar(yt[j][:], ytr[:], aff_a[:, 0:1],
                                        aff_b[:, 0:1],
                                        op0=ALU.mult, op1=ALU.add)

            def new_head(j):
                head_ctx[j] = (
                    wp.tile([128, T], BF16, tag="ytr", name="ytr", bufs=3),
                    wp.tile([128, 4], F32, tag="s1p", name="s1p"),
                    wp.tile([128, 4], F32, tag="s2p", name="s2p"))

            # qb order (1,2,3,0): the serial end-of-pair chain (last exp ->
            # combine -> transpose -> z -> stats -> gn) runs on the smallest
            # q-block, shrinking the pair-boundary latency
            QBS = (1, 2, 3, 0)

            # ---- pair (0,1): bg (v + oc1 projections) fills the PE ----
            for j in (0, 1):
                new_head(j)
            AB23_first = None
            for qb in QBS:
                ABs = phase1_pair(0, 1, qb)
                bg_drain(6)
                if qb == 0:
                    # boundary interleave: emit pair-23's first (big) S/exp
                    # block now so ACT/DVE stay fed while the PE chews the
                    # remaining qb0 z-matmuls, gn chains and fills
                    bg_drain(len(bg))
                    AB23_first = phase1_pair(2, 3, QBS[0])
                phase2(0, qb, ABs[0])
                if qb == 0:
                    gn_final(0)
                phase2(1, qb, ABs[1])
                if qb == 0:
                    gn_final(1)

            # pair boundary: release x/weight tiles, open the partial pool
            lp_cm.__exit__(None, None, None)
            dp_cm = tc.tile_pool(name="drain", bufs=1)
            dp = dp_cm.__enter__()
            p01 = [dp.tile([128, T], BF16, tag=f"p01_{ocb}",
                           name=f"p01_{ocb}") for ocb in range(8)]

            # out-proj j=0,1 partials: fill PE during the ACT-bound
            # (2,3) pair. Two ocbs per qb slot.
            fill_q = list(range(8))

            def emit_fill(ocb):
                for tb in range(T // 512):
                    pt = ps_t.tile([128, 512], F32, tag="tz", name="fl")
                    for j in (0, 1):
                        nc.tensor.matmul(
                            pt[:],
                            wct[j][:, ocb * 128:(ocb + 1) * 128],
                            yt[j][:, tb * 512:(tb + 1) * 512],
                            start=(j == 0), stop=(j == 1),
                            skip_group_check=True)
                    nc.vector.tensor_copy(
                        p01[ocb][:, tb * 512:(tb + 1) * 512], pt[:])
                # ship the j01 partial to the host during the attention
                # phase (DMA engines are idle here); host adds the partials
                nc.gpsimd.dma_start(
                    outT2_d[ocb * 128:(ocb + 1) * 128, :], p01[ocb][:])

            # ---- pair (2,3): fills + attention ----
            for j in (2, 3):
                new_head(j)
            fill_budget = {1: 2, 2: 2, 3: 1, 0: 2}
            for qb in QBS:
                ABs = (AB23_first if qb == QBS[0]
                       else phase1_pair(2, 3, qb))
                for _ in range(fill_budget[qb]):
                    if fill_q:
                        # at qb==0 this covers the last transpose-chain
                        # latency before z(2,0)
                        emit_fill(fill_q.pop(0))
                phase2(2, qb, ABs[2])
                if qb == 0:
                    # the held-back fill gives the PE gn-independent work
                    # while the gn chains run
                    gn_final(2)
                    if fill_q:
                        emit_fill(fill_q.pop(0))
                phase2(3, qb, ABs[3])
                if qb == 0:
                    gn_final(3)
                    while fill_q:
                        emit_fill(fill_q.pop(0))

            # ================= output projection =================
            # per ocb: ident-add of the j01 partial, then j=2, then j=3.
            # gn(3) is emitted just before ocb0 so its ACT->DVE chain hides
            # behind the gn-independent ident-adds + j=2 matmuls.
            def s_halves():
                psA = ps_s.tile([128, 1024], F32, tag="s", name="psA")
                psB = ps_s.tile([128, 1024], F32, tag="s", name="psB")
                return (psA, psB)

            def op_mms23(ocb, halves):
                for j in (2, 3):
                    for tb in range(T // 512):
                        ph = halves[tb // 2]
                        mi = nc.tensor.matmul(
                            ph[:, (tb % 2) * 512:(tb % 2) * 512 + 512],
                            wct[j][:, ocb * 128:(ocb + 1) * 128],
                            yt[j][:, tb * 512:(tb + 1) * 512],
                            start=(j == 2), stop=(j == 3),
                            skip_group_check=True)
                        if tb > 0:
                            mi.ins.ldweights = False

            _fin_rr = [0]

            def op_fin(ocb, halves):
                for tb2 in range(2):
                    ob = dp.tile([128, 1024], BF16, tag="ob", bufs=4,
                                 name="ob")
                    r = _fin_rr[0] = (_fin_rr[0] + 1) % 2
                    if r == 0:
                        nc.vector.tensor_copy(ob[:], halves[tb2][:])
                    else:
                        nc.scalar.copy(ob[:], halves[tb2][:])
                    eng = (nc.sync, nc.gpsimd, nc.scalar)[(2 * ocb + tb2) % 3]
                    eng.dma_start(
                        outT_d[ocb * 128:(ocb + 1) * 128,
                               tb2 * 1024:(tb2 + 1) * 1024], ob[:])

            for ocb in range(8):
                halves = s_halves()
                op_mms23(ocb, halves)
                op_fin(ocb, halves)
            dp_cm.__exit__(None, None, None)
            wp_cm.__exit__(None, None, None)

    nc.compile()
    return nc


def _prep_inputs(inputs):
    bf = ml_dtypes.bfloat16
    x = np.asarray(inputs["x"], np.float32)
    Wq1 = np.asarray(inputs["Wq1"], np.float32)
    Wq2 = np.asarray(inputs["Wq2"], np.float32)
    Wk1 = np.asarray(inputs["Wk1"], np.float32)
    Wk2 = np.asarray(inputs["Wk2"], np.float32)
    Wv = np.asarray(inputs["Wv"], np.float32)
    Wc = np.asarray(inputs["Wc"], np.float32)
    gn_w = np.asarray(inputs["gn_w"], np.float32)
    gn_b = np.asarray(inputs["gn_b"], np.float32)
    gamma = np.asarray(inputs["gamma"], np.float32)

    def sig(v):
        return 1.0 / (1.0 + np.exp(-v))

    lam = (sig(np.asarray(inputs["lq1"], np.float32).reshape(H)
               * np.asarray(inputs["lk1"], np.float32).reshape(H))
           - sig(np.asarray(inputs["lq2"], np.float32).reshape(H)
                 * np.asarray(inputs["lk2"], np.float32).reshape(H))
           + LAMBDA_INIT)

    mask = np.where(np.arange(128)[None, :] <= np.arange(128)[:, None],
                    0.0, NEG).astype(bf)
    ident = np.eye(128, dtype=np.float32).astype(bf)
    gg = (np.arange(128)[:, None] // 4 == np.arange(128)[None, :] // 4
          ).astype(np.float32)
    c1 = 1.0 - LAMBDA_INIT
    gw2 = (gn_w * gamma * c1).astype(np.float32).reshape(128, 1)
    gb2 = (gn_b * gamma * c1).astype(np.float32).reshape(128, 1)

    icon = np.zeros((128, 2), np.uint32)
    icon[:, 0] = 1
    icon[:, 1] = 0x5f375a00
    cstb = np.concatenate([mask, ident], axis=1)
    xTb = [np.ascontiguousarray(x[b].T).astype(bf) for b in range(B)]
    in_maps = []
    for core in range(N_CORES):
        b, hg = divmod(core, N_CORES // B)
        qs = hg * NH * HS          # 256-wide q/k slice
        vs = hg * NH * D           # 512-wide v / y2 slice
        lamn = np.repeat(-lam[hg * NH:(hg + 1) * NH].reshape(1, NH),
                         128, axis=0).astype(np.float32)
        cstf = np.concatenate(
            [gg, gw2, gb2, icon.view(np.float32), lamn],
            axis=1).astype(np.float32)
        def mtiles(wt, cols):
            # [1024, cols] -> merged 2-tile layout [256, 4*cols]
            return np.ascontiguousarray(
                wt.reshape(2, 4, 128, cols).transpose(0, 2, 1, 3)
                .reshape(256, 4 * cols)).astype(bf)

        in_maps.append({
            "xT": xTb[b],
            "wq1T": mtiles(Wq1[qs:qs + NH * HS, :].T, NH * HS),
            "wq2T": mtiles(Wq2[qs:qs + NH * HS, :].T, NH * HS),
            "wk1T": mtiles(Wk1[qs:qs + NH * HS, :].T, NH * HS),
            "wk2T": mtiles(Wk2[qs:qs + NH * HS, :].T, NH * HS),
            "wvT": mtiles(Wv[vs:vs + NH * D, :].T, NH * D),
            "wcT": np.ascontiguousarray(Wc[:, vs:vs + NH * D].T).astype(bf),
            "cstf": cstf,
            "cstb": cstb,
        })
    return in_maps


def kernel(**inputs):
    if "nc" not in _cache:
        _cache["nc"] = _build()
    nc = _cache["nc"]
    in_maps = _prep_inputs(inputs)
    res = bass_utils.run_bass_kernel_spmd(
        nc, in_maps, core_ids=list(range(N_CORES)),
        **_cache.get("run_kwargs", {}))
    _cache["last_result"] = res
    out = np.zeros((B, T, C), np.float32)
    for core in range(N_CORES):
        b = core // (N_CORES // B)
        out[b] += res.results[core]["outT"].T.astype(np.float32)
        out[b] += res.results[core]["outT2"].T.astype(np.float32)
    return out
